# revision 5
# baseline (speedup 1.0000x reference)
"""Self-contained BEVNet dilated-neighborhood-attention kernel for 8 Trainium2
NeuronCores (Bass/Tile NEFF, one shard per core, single cached PJRT dispatch).

Sharding: 8 shards = batch(2) x row-quarters(4 x 40 rows). Each core gets its
40-row slab (bf16) plus a 2-row halo, computes qkv -> 3x3 dilated neighborhood
attention (dilations 1 and 2, 4 heads each) -> proj, returns bf16 [6400, 128].

See build_bass() docstring below for the on-chip layout.
"""

import json as _json

import numpy as np
import ml_dtypes

import jax
import jax.numpy as jnp
from jax.experimental.shard_map import shard_map
from jax.sharding import Mesh, NamedSharding, PartitionSpec as P

import concourse.bass as bass
import concourse.mybir as mybir
import concourse.tile as tile
from concourse import bass2jax

BF16 = mybir.dt.bfloat16
F32 = mybir.dt.float32
NPBF16 = ml_dtypes.bfloat16

W = 160
WP = 164             # width padded by 2 zero cols each side
ROWS = 40            # active rows per shard
SLAB = 44            # stored rows (2 halo each side)
PIX = SLAB * WP      # 7216
HS = 24 * WP         # 3936 stored pixels per half (rows 0..23 / 20..43)
HB_OFF = 20 * WP     # xT offset of half B (slab row 20)
ACT_OFF = 2 * WP     # active-row offset within a half
ASPAN = 20 * WP      # 3280 active pixels per half
NOUT = ROWS * W      # 6400 real output pixels per shard
DILS = (1, 2)
QKV_CHUNK = 492      # 8 * 492 = HS
SC_CHUNK = 410       # 8 * 410 = ASPAN
HS4 = HS + 4         # stored half + 2-pixel guard each end
QOFF = 2
NB = 9
NCORE = 8

# engine-assignment tunables (index-based predicates)
CFG = {
    "qkv_copy_dve": lambda wi, g, c8: False,     # True -> DVE else ACT
    "prod_pool": lambda g, sh, nb: False,        # True -> Pool else DVE
    "ebig_copy_dve": lambda cg, g, nb, c: False, # True -> DVE else ACT
    "vmul_pool": lambda cg, g, nb: False,        # True -> Pool else DVE
    "add_pool": lambda cg, g, nb: nb in (4, 8),  # True -> Pool else DVE
    "vmul_psum": lambda cg, g, nb: False,        # True -> skip ebig copy,
                                                 # DVE mul reads PSUM at 1x
}


def build_consts():
    # segment-sum: per nb [128, 72]: sseg[p, nb*72 + 8*nb + p//16] = 1
    ones8 = np.zeros((128, NB * 72), dtype=NPBF16)
    for nb in range(NB):
        for p in range(128):
            ones8[p, nb * 72 + 8 * nb + p // 16] = 1
    # den-sum: [72, 8] dsum[8*nb + m, m] = 1
    dsum = np.zeros((72, 8), dtype=NPBF16)
    for nb in range(NB):
        for m in range(8):
            dsum[8 * nb + m, m] = 1
    # expand: per nb [72, 128]: exp[8*nb + c//16, nb*128 + c] = 1
    expand = np.zeros((72, NB * 128), dtype=NPBF16)
    for nb in range(NB):
        for c in range(128):
            expand[8 * nb + c // 16, nb * 128 + c] = 1
    # r-expand: [8, 128] rexp[c//16, c] = 1
    rexp = np.zeros((8, 128), dtype=NPBF16)
    for c in range(128):
        rexp[c // 16, c] = 1
    # expanded segment-sum: [128, 128] sexp[p, c] = 1 if p//16 == c//16
    sexp = np.zeros((128, 128), dtype=NPBF16)
    for p in range(128):
        for c in range(128):
            if p // 16 == c // 16:
                sexp[p, c] = 1
    # den from expanded E: [128, 8] dsum2[p, m] = (1/16) if p//16 == m
    dsum2 = np.zeros((128, 8), dtype=NPBF16)
    for p in range(128):
        dsum2[p, p // 16] = 1.0 / 16
    return ones8, dsum, expand, rexp, sexp, dsum2


def _split_multi_waits(bir):
    """This walrus build accepts at most ONE sync-wait command per
    instruction; hoist extra waits onto standalone EventSemaphore ops
    (same engine stream, identical blocking semantics)."""
    n = 0
    for f in bir["functions"]:
        for bb in f["blocks"]:
            new_insts = []
            for inst in bb["instructions"]:
                si = inst.get("sync_info")
                w = (si or {}).get("on_wait") or []
                if len(w) > 1:
                    for extra in w[:-1]:
                        n += 1
                        new_insts.append({
                            "debug": inst.get("debug", 0),
                            "engine": inst["engine"],
                            "ins": [], "outs": [],
                            "name": f"WSPLIT-{n}",
                            "opcode": "EventSemaphore",
                            "sync_info": {"on_update": [],
                                          "on_wait": [extra]},
                        })
                    si["on_wait"] = [w[-1]]
                new_insts.append(inst)
            bb["instructions"] = new_insts
    return bir


def build_bass():
    """Per-core program. Channels-on-partitions, pixels-on-free, with each
    dilation group's q/k/v stored pixel-halved ([64ch x 2 row-halves] on
    partitions) so every DVE elementwise op runs all 128 lanes with one
    uniform free-dim shift per (group, neighbor)."""
    nc = bass.Bass()

    x_main = nc.dram_tensor("x_main", [NOUT, 128], BF16, kind="ExternalInput")
    x_halo = nc.dram_tensor("x_halo", [4 * W, 128], BF16, kind="ExternalInput")
    wqkv = nc.dram_tensor("wqkv", [128, 384], BF16, kind="ExternalInput")
    wproj = nc.dram_tensor("wproj", [128, 256], BF16, kind="ExternalInput")
    ones8_d = nc.dram_tensor("ones8", [128, NB * 72], BF16, kind="ExternalInput")
    dsum_d = nc.dram_tensor("dsum", [72, 8], BF16, kind="ExternalInput")
    expand_d = nc.dram_tensor("expand", [72, NB * 128], BF16, kind="ExternalInput")
    rexp_d = nc.dram_tensor("rexp", [8, 128], BF16, kind="ExternalInput")
    sexp_d = nc.dram_tensor("sexp", [128, 128], BF16, kind="ExternalInput")
    dsum2_d = nc.dram_tensor("dsum2", [128, 8], BF16, kind="ExternalInput")
    pbias_d = nc.dram_tensor("pbias", [128, 1], F32, kind="ExternalInput")
    y_out = nc.dram_tensor("y", [NOUT, 128], BF16, kind="ExternalOutput")

    with nc.allow_low_precision(reason="bf16 attention; tolerance 2e-2"), \
         tile.TileContext(nc) as tc:
        with tc.tile_pool(name="consts", bufs=1) as consts, \
             tc.tile_pool(name="big", bufs=1) as big, \
             tc.tile_pool(name="work", bufs=3) as work, \
             tc.tile_pool(name="pmm", bufs=2, space="PSUM") as pmm, \
             tc.tile_pool(name="psc", bufs=3, space="PSUM") as psc, \
             tc.tile_pool(name="pex", bufs=3, space="PSUM") as pex:

            wqkv_sb = consts.tile([128, 384], BF16, name="wqkv_sb")
            nc.sync.dma_start(out=wqkv_sb, in_=wqkv[:, :])
            wproj_sb = consts.tile([128, 256], BF16, name="wproj_sb")
            nc.sync.dma_start(out=wproj_sb, in_=wproj[:, :])
            ones8_sb = consts.tile([128, NB * 72], BF16, name="ones8_sb")
            nc.sync.dma_start(out=ones8_sb, in_=ones8_d[:, :])
            dsum_sb = consts.tile([72, 8], BF16, name="dsum_sb")
            nc.sync.dma_start(out=dsum_sb, in_=dsum_d[:, :])
            expand_sb = consts.tile([72, NB * 128], BF16, name="expand_sb")
            nc.sync.dma_start(out=expand_sb, in_=expand_d[:, :])
            rexp_sb = consts.tile([8, 128], BF16, name="rexp_sb")
            nc.sync.dma_start(out=rexp_sb, in_=rexp_d[:, :])
            sexp_sb = consts.tile([128, 128], BF16, name="sexp_sb")
            nc.sync.dma_start(out=sexp_sb, in_=sexp_d[:, :])
            dsum2_sb = consts.tile([128, 8], BF16, name="dsum2_sb")
            nc.sync.dma_start(out=dsum2_sb, in_=dsum2_d[:, :])
            pbias_sb = consts.tile([128, 1], F32, name="pbias_sb")
            nc.sync.dma_start(out=pbias_sb, in_=pbias_d[:, :])

            # ---- xT [128 ch, PIX] via DMA xbar transpose
            xT = big.tile([128, PIX], BF16, name="xT")
            xT3 = xT.rearrange("p (r w) -> p r w", w=WP)
            nc.vector.memset(xT3[:, :, 0:2], 0.0)
            nc.vector.memset(xT3[:, :, 162:164], 0.0)
            xstage = big.tile([128, NOUT + 4 * W], BF16, tag="stage", name="xstage")
            for q in range(4):
                nc.sync.dma_start_transpose(
                    out=xstage[:, 1600 * q:1600 * (q + 1)],
                    in_=x_main[1600 * q:1600 * (q + 1), :])
            nc.sync.dma_start_transpose(
                out=xstage[:, NOUT:NOUT + 4 * W], in_=x_halo[:, :])
            xs3 = xstage[:, 0:NOUT].rearrange("p (r w) -> p r w", w=W)
            for q in range(4):
                nc.vector.tensor_copy(xT3[:, 2 + 10 * q:12 + 10 * q, 2:162],
                                      xs3[:, 10 * q:10 * (q + 1), :])
            hs3 = xstage[:, NOUT:NOUT + 4 * W].rearrange(
                "p (r w) -> p r w", w=W)
            nc.vector.tensor_copy(xT3[:, 0:2, 2:162], hs3[:, 0:2, :])
            nc.vector.tensor_copy(xT3[:, 42:44, 2:162], hs3[:, 2:4, :])

            # ---- qkv into pixel-halved per-group buffers [128, HS4]
            qT = [big.tile([128, HS4], BF16, name=f"qT{g}") for g in range(2)]
            kT = [big.tile([128, HS4], BF16, name=f"kT{g}") for g in range(2)]
            vT = [big.tile([128, HS4], BF16, name=f"vT{g}") for g in range(2)]
            dests = [qT, kT, vT]
            for g in range(2):
                for t in (kT[g], vT[g]):
                    nc.vector.memset(t[:, 0:QOFF], 0.0)
                    nc.vector.memset(t[:, QOFF + HS:HS4], 0.0)
            for g in range(2):
                for wi in range(3):
                    lhsT = wqkv_sb[:, (wi * 2 + g) * 64:(wi * 2 + g + 1) * 64]
                    dest = dests[wi][g]
                    for c8 in range(8):
                        ps = pmm.tile([128, QKV_CHUNK], F32, tag="mm",
                                      name="ps_mm")
                        nc.tensor.matmul(
                            ps[0:64, :], lhsT,
                            xT[:, QKV_CHUNK * c8:QKV_CHUNK * (c8 + 1)],
                            start=True, stop=True)
                        nc.tensor.matmul(
                            ps[64:128, :], lhsT,
                            xT[:, HB_OFF + QKV_CHUNK * c8:
                               HB_OFF + QKV_CHUNK * (c8 + 1)],
                            start=True, stop=True, tile_position=(0, 64))
                        dsl = dest[:, QOFF + QKV_CHUNK * c8:
                                   QOFF + QKV_CHUNK * (c8 + 1)]
                        if CFG["qkv_copy_dve"](wi, g, c8):
                            nc.vector.tensor_copy(dsl, ps)
                        else:
                            nc.scalar.copy(out=dsl, in_=ps)

            offs = [[r * (dy * WP + dx)
                     for dy in (-1, 0, 1) for dx in (-1, 0, 1)] for r in DILS]

            # ---- scores -> E = exp(scores), rr = 1/sum(E)
            E = [big.tile([72, ASPAN], BF16, name=f"E{g}") for g in range(2)]
            rr = [big.tile([8, ASPAN], BF16, name=f"r{g}") for g in range(2)]
            s0a = QOFF + ACT_OFF
            HSPAN = ASPAN // 2
            o_acc = [big.tile([128, ASPAN], BF16, name=f"oacc{g}")
                     for g in range(2)]
            yT = big.tile([128, NOUT], BF16, name="yT")

            def sv_block(g, sh):
                b0 = HSPAN * sh
                prods = []
                for nb in range(NB):
                    off = offs[g][nb]
                    prod = work.tile([128, HSPAN], BF16, tag="prod",
                                     name="prod", bufs=9)
                    eng_p = (nc.gpsimd if CFG["prod_pool"](g, sh, nb)
                             else nc.vector)
                    eng_p.tensor_mul(
                        prod, qT[g][:, s0a + b0:s0a + b0 + HSPAN],
                        kT[g][:, s0a + b0 + off:s0a + b0 + off + HSPAN])
                    prods.append(prod)
                ebigs = []
                for nb in range(NB):
                    ebig = work.tile([128, HSPAN], BF16, tag="ebig",
                                     name="ebig", bufs=10)
                    for c in range(4):
                        a0 = SC_CHUNK * c
                        pe_ = pex.tile([128, SC_CHUNK], F32, tag="ex",
                                       name="ps_ex")
                        nc.tensor.matmul(pe_, sexp_sb,
                                         prods[nb][:, a0:a0 + SC_CHUNK],
                                         start=True, stop=True)
                        nc.scalar.activation(
                            out=ebig[:, a0:a0 + SC_CHUNK], in_=pe_,
                            func=mybir.ActivationFunctionType.Exp)
                    ebigs.append(ebig)
                # den + recip per chunk
                for c in range(4):
                    a0 = SC_CHUNK * c
                    pd = psc.tile([128, SC_CHUNK], F32, tag="sc",
                                  name="ps_den")[0:8, :]
                    for nb in range(NB):
                        nc.tensor.matmul(pd, dsum2_sb,
                                         ebigs[nb][:, a0:a0 + SC_CHUNK],
                                         start=(nb == 0), stop=(nb == NB - 1))
                    nc.vector.reciprocal(
                        out=rr[g][:, b0 + a0:b0 + a0 + SC_CHUNK], in_=pd)
                # weighted-v accumulation
                oslice = o_acc[g][:, b0:b0 + HSPAN]
                for nb in range(NB):
                    off = offs[g][nb]
                    vsrc = vT[g][:, s0a + b0 + off:s0a + b0 + off + HSPAN]
                    eng_m = (nc.gpsimd if CFG["vmul_pool"](sh, g, nb)
                             else nc.vector)
                    if nb == 0:
                        eng_m.tensor_mul(oslice, ebigs[nb], vsrc)
                    else:
                        ev = work.tile([128, HSPAN], BF16, tag="ev",
                                       name="ev", bufs=3)
                        eng_m.tensor_mul(ev, ebigs[nb], vsrc)
                        eng_a = (nc.gpsimd if CFG["add_pool"](sh, g, nb)
                                 else nc.vector)
                        eng_a.tensor_add(oslice, oslice, ev)

            def np_block(cg):
                cg0 = HSPAN * cg
                for g in range(2):
                    for c in range(4):
                        a0 = cg0 + SC_CHUNK * c
                        rb = psc.tile([128, SC_CHUNK], F32, tag="sc",
                                      name="ps_rb")
                        nc.tensor.matmul(rb, rexp_sb,
                                         rr[g][:, a0:a0 + SC_CHUNK],
                                         start=True, stop=True)
                        nc.vector.tensor_mul(o_acc[g][:, a0:a0 + SC_CHUNK],
                                             o_acc[g][:, a0:a0 + SC_CHUNK],
                                             rb)
                for half in range(2):
                    for q10 in range(5 * cg, 5 * (cg + 1)):
                        ps = pmm.tile([128, 320], F32, tag="mm", name="ps_y")
                        rhs = []
                        for g in range(2):
                            o3 = o_acc[g][64 * half:64 * (half + 1), :] \
                                .rearrange("p (r w) -> p r w", w=WP)
                            rhs.append(o3[:, 2 * q10:2 * q10 + 2, 2:162])
                        base = 64 * half
                        nc.tensor.matmul(ps, wproj_sb[base:base + 64, 0:128],
                                         rhs[0], start=True, stop=False)
                        nc.tensor.matmul(ps,
                                         wproj_sb[base:base + 64, 128:256],
                                         rhs[1], start=False, stop=True)
                        nc.scalar.activation(
                            out=yT[:, 3200 * half + 320 * q10:
                                   3200 * half + 320 * (q10 + 1)],
                            in_=ps,
                            func=mybir.ActivationFunctionType.Identity,
                            bias=pbias_sb[:, 0:1], scale=1.0)

            for sh in range(2):
                for g in range(2):
                    sv_block(g, sh)
                np_block(sh)

            # ---- transpose back and store
            ynat = big.tile([128, NOUT + 4 * W], BF16, tag="stage", name="ynat")[:, 0:NOUT]
            ynat3 = ynat.rearrange("p (t c) -> p t c", c=128)
            for t in range(NOUT // 128):
                nc.sync.dma_start_transpose(
                    out=ynat3[:, t, :], in_=yT[:, 128 * t:128 * (t + 1)])
            y3 = y_out.rearrange("(t p) c -> p t c", p=128)
            for s in range(5):
                nc.sync.dma_start(out=y3[:, 10 * s:10 * (s + 1), :],
                                  in_=ynat3[:, 10 * s:10 * (s + 1), :])

    raw = bass.Bass.to_json_bytes(nc)
    fixed = _json.dumps(_split_multi_waits(_json.loads(raw))).encode()
    nc.to_json_bytes = lambda: fixed
    return nc


# ---------------- host side ----------------

def prep_weights(qkv_w, proj_w, proj_b):
    scale = 16 ** -0.5
    qw = np.asarray(qkv_w, dtype=np.float32).copy()
    qw[0:128] *= scale
    wqkv = np.ascontiguousarray(qw.T).astype(NPBF16)
    wpT = np.ascontiguousarray(np.asarray(proj_w, dtype=np.float32).T)
    wproj = np.zeros((128, 256), dtype=np.float32)
    wproj[0:64, 0:128] = wpT[0:64]
    wproj[0:64, 128:256] = wpT[64:128]
    wproj[64:128] = wproj[0:64]
    wproj = wproj.astype(NPBF16)
    pbias = np.asarray(proj_b, dtype=np.float32).reshape(128, 1)
    return wqkv, wproj, pbias


def host_inputs(x, qkv_w, proj_w, proj_b):
    """Full inputs -> dict name -> globally-concatenated (8*dim0) arrays."""
    xb = np.asarray(x, dtype=np.float32).astype(NPBF16)
    x_main = np.ascontiguousarray(xb.reshape(NCORE * NOUT, 128))
    halos = np.zeros((NCORE, 4 * W, 128), dtype=NPBF16)
    for idx in range(NCORE):
        b, j = divmod(idx, 4)
        if j > 0:
            halos[idx, 0:2 * W] = xb[b, 40 * j - 2:40 * j].reshape(2 * W, 128)
        if j < 3:
            halos[idx, 2 * W:4 * W] = \
                xb[b, 40 * j + 40:40 * j + 42].reshape(2 * W, 128)
    wqkv, wproj, pbias = prep_weights(qkv_w, proj_w, proj_b)
    ones8, dsum, expand, rexp, sexp, dsum2 = build_consts()
    reps = {"wqkv": wqkv, "wproj": wproj, "pbias": pbias, "ones8": ones8,
            "dsum": dsum, "expand": expand, "rexp": rexp, "sexp": sexp,
            "dsum2": dsum2}
    g = {"x_main": x_main, "x_halo": halos.reshape(NCORE * 4 * W, 128)}
    for k, v in reps.items():
        g[k] = np.ascontiguousarray(
            np.broadcast_to(v[None], (NCORE,) + v.shape)
            .reshape((NCORE * v.shape[0],) + v.shape[1:]))
    return g


_STATE = {}


def _get_runner():
    if "runner" in _STATE:
        return _STATE["runner"]

    bass2jax.install_neuronx_cc_hook()
    nc = build_bass()

    partition_name = (nc.partition_id_tensor.name
                      if nc.partition_id_tensor else None)
    in_names, out_names, out_avals, zero_shapes = [], [], [], []
    for alloc in nc.m.functions[0].allocations:
        if not isinstance(alloc, mybir.MemoryLocationSet):
            continue
        name = alloc.memorylocations[0].name
        if alloc.kind == "ExternalInput":
            if name == partition_name:
                continue
            in_names.append(name)
        elif alloc.kind == "ExternalOutput":
            out_names.append(name)
            shape = tuple(alloc.tensor_shape)
            dtype = mybir.dt.np(alloc.dtype)
            out_avals.append(jax.core.ShapedArray(shape, dtype))
            zero_shapes.append((shape, dtype))
    n_params = len(in_names)
    n_outs = len(out_names)
    all_names = in_names + out_names
    if partition_name is not None:
        all_names = all_names + [partition_name]

    def _body(*args):
        operands = list(args)
        if partition_name is not None:
            operands.append(bass2jax.partition_id_tensor())
        outs = bass2jax._bass_exec_p.bind(
            *operands,
            out_avals=tuple(out_avals),
            in_names=tuple(all_names),
            out_names=tuple(out_names),
            lowering_input_output_aliases=(),
            sim_require_finite=True,
            sim_require_nnan=True,
            nc=nc)
        return tuple(outs)

    devices = jax.devices()[:NCORE]
    mesh = Mesh(np.asarray(devices), ("core",))
    in_specs = (P("core"),) * (n_params + n_outs)
    out_specs = (P("core"),) * n_outs
    donate = tuple(range(n_params, n_params + n_outs))
    sharded = jax.jit(
        shard_map(_body, mesh=mesh, in_specs=in_specs, out_specs=out_specs,
                  check_rep=False),
        donate_argnums=donate, keep_unused=True)

    def zeros_fn():
        return tuple(
            jnp.zeros((NCORE * s[0],) + tuple(s[1:]), d)
            for s, d in zero_shapes)
    zfn = jax.jit(
        zeros_fn,
        out_shardings=tuple(NamedSharding(mesh, P("core"))
                            for _ in zero_shapes))

    def runner(global_in: dict):
        args = [global_in[n] for n in in_names]
        zeros = zfn()
        outs = sharded(*args, *zeros)
        return dict(zip(out_names, outs))

    _STATE["runner"] = runner
    _STATE["nc"] = nc
    return runner


def kernel(x, qkv_w, proj_w, proj_b):
    runner = _get_runner()
    gin = host_inputs(x, qkv_w, proj_w, proj_b)
    outs = runner(gin)
    y = np.asarray(outs["y"])          # [8*NOUT, 128] bf16
    return y.reshape(2, 160, 160, 128).astype(np.float32)
